# revision 1
# baseline (speedup 1.0000x reference)
"""Trainium2 Bass kernel for nn_BSplineLayer (B-spline control-point solve + curve eval).

Key insight: the whole reference computation is LINEAR in the input radii r:
  Q = A @ r          (control-point solve: weighted sums + two first-order
                      linear recursions -> a dense 64x64 matrix A)
  curve = T @ Q      (closed cubic B-spline eval: per-segment gather of 4
                      control points x cubic basis -> sparse 1260x63 matrix T)
so  out[b, m, 0, c] = sum_n G[m, n] * r[b, n, c]  with  G = T @ A  (1260x64),
precomputed on the host in float64.

On device (per core, pure data parallel over batch):
  - load x tile [128 batch, 128] (c-major: col = c*64+n, host pre-permuted)
  - PE transpose -> rT [c*64+n, batch] in PSUM -> SBUF
  - PE matmul:  out[b, col] = rT.T @ GB  with GB [128, 2520] embedding G for
    both channels with interleaved output columns (col = 2m+c)
  - evacuate PSUM -> SBUF via ScalarE/VectorE, DMA out [128, 2520] contiguous.

The kernel is memory-bound: ~20.7 MB of HBM traffic per core (output dominates),
floor ~58 us at ~358 GB/s per-core HBM bandwidth.
"""

import os

import numpy as np

import concourse.bacc as bacc
import concourse.mybir as mybir
import concourse.tile as tile
from concourse.bass import ts
from concourse.bass_utils import run_bass_kernel_spmd

# Problem shape (hardcoded per contract: kernel.py is self-contained).
B, N, C = 16384, 64, 2
NCORES = 8
BPC = B // NCORES          # 2048 batch rows per core
P = 128                    # SBUF partitions
NTILES = BPC // P          # 16 batch tiles per core
NSEG = N - 1               # 63 segments
SAMP = 20                  # samples per segment
MOUT = NSEG * SAMP         # 1260 curve points
FIN = N * C                # 128 input floats per batch row
FOUT = MOUT * C            # 2520 output floats per batch row

# mode: "f32r_wide" (fp32 data, PE fp32r fast path, K=128 zero-interleaved G)
#       "fp32_wide" (exact fp32 matmul, 4x slower PE)
#       "fp32_packed" / "f32r_packed" (two K=64 row-group matmuls per tile)
MODE = os.environ.get("BSPLINE_MODE", "f32r_wide")
TRACE = bool(int(os.environ.get("BSPLINE_TRACE", "0")))

LAST_RESULT = None  # BassKernelResults of the most recent run (for test harness)


def _build_G(dtype=np.float64) -> np.ndarray:
    """G [1260, 64]: out[b, m, c] = sum_n G[m, n] * r[b, n, c]."""
    z1 = -2.0 + np.sqrt(np.asarray(3.0, dtype=dtype))
    powers = z1 ** np.arange(N, dtype=dtype)
    denom = 1.0 - z1**N
    # QT[i] as a linear functional of r (rows of a matrix); the *255/255
    # scaling in the reference cancels by linearity.
    QT = np.zeros((N, N), dtype=dtype)
    QT[0] = powers / denom
    for i in range(1, N):
        QT[i] = z1 * QT[i - 1]
        QT[i, i] += 1.0
    A = np.zeros((N, N), dtype=dtype)
    A[0] = -(6.0 * z1 / denom) * (powers[:, None] * QT).sum(axis=0)
    A[N - 1] = z1 * A[0] - 6.0 * z1 * QT[N - 1]
    for i in range(N - 2, 0, -1):
        A[i] = z1 * A[i + 1] - 6.0 * z1 * QT[i]
    # Cubic B-spline basis: curve[m=seg*20+s] = sum_k W[k, s] * Q[(seg+k) % 63]
    M = np.array(
        [
            [-1 / 6, 0.5, -0.5, 1 / 6],
            [0.5, -1.0, 0.5, 0.0],
            [-0.5, 0.0, 0.5, 0.0],
            [1 / 6, 2 / 3, 1 / 6, 0.0],
        ],
        dtype=dtype,
    )
    s = np.linspace(0.0, 1.0, SAMP).astype(dtype)
    S = np.stack([s**3, s**2, s, np.ones_like(s)], axis=0)
    W = M.T @ S  # [4, 20]
    G = np.zeros((MOUT, N), dtype=dtype)
    for seg in range(NSEG):
        for k in range(4):
            G[seg * SAMP : (seg + 1) * SAMP, :] += (
                W[k][:, None] * A[(seg + k) % NSEG][None, :]
            )
    return G


def _g_const(mode: str) -> np.ndarray:
    G = _build_G().astype(np.float32)
    if mode.endswith("wide"):
        # GB[c*64+n, 2m+c] = G[m, n]; zero elsewhere (K=128 single matmul).
        GB = np.zeros((P, FOUT), dtype=np.float32)
        for c in range(C):
            GB[c * N : (c + 1) * N, c::2] = G.T
        return GB
    # packed: GD[c*64+n, m] = G[m, n] (duplicated for both row groups).
    return np.concatenate([G.T, G.T], axis=0).astype(np.float32)


def _build_nc(mode: str):
    f32 = mybir.dt.float32
    f32r = mybir.dt.float32r
    use_f32r = mode.startswith("f32r")
    gcols = FOUT if mode.endswith("wide") else MOUT

    nc = bacc.Bacc("TRN2", target_bir_lowering=False, debug=False, num_devices=NCORES)
    x = nc.dram_tensor("x", [BPC, FIN], f32, kind="ExternalInput").ap()
    g = nc.dram_tensor("g", [P, gcols], f32, kind="ExternalInput").ap()
    ident = nc.dram_tensor("ident", [P, P], f32, kind="ExternalInput").ap()
    out = nc.dram_tensor("out", [BPC, FOUT], f32, kind="ExternalOutput").ap()

    def mmdt(ap):
        return ap.bitcast(f32r) if use_f32r else ap

    with tile.TileContext(nc) as tc:
        with (
            tc.tile_pool(name="const", bufs=1) as cpool,
            tc.tile_pool(name="xin", bufs=3) as xpool,
            tc.tile_pool(name="rt", bufs=2) as rpool,
            tc.tile_pool(name="outs", bufs=3) as opool,
            tc.tile_pool(name="pst", bufs=2, space="PSUM") as pst,
            tc.tile_pool(name="pso", bufs=6, space="PSUM") as pso,
        ):
            g_sb = cpool.tile([P, gcols], f32)
            nc.sync.dma_start(g_sb[:], g[:])
            id_sb = cpool.tile([P, P], f32)
            nc.sync.dma_start(id_sb[:], ident[:])

            for t in range(NTILES):
                xt = xpool.tile([P, FIN], f32)
                nc.sync.dma_start(xt[:], x[ts(t, P), :])
                pt = pst.tile([P, P], f32)
                nc.tensor.transpose(mmdt(pt[:]), mmdt(xt[:]), mmdt(id_sb[:]))
                rt = rpool.tile([P, P], f32)
                nc.vector.tensor_copy(rt[:], pt[:])

                if mode.endswith("wide"):
                    CH = 504  # 5 chunks x 504 = 2520; one PSUM bank each
                    ot = opool.tile([P, FOUT], f32)
                    for j in range(FOUT // CH):
                        lo = j * CH
                        pj = pso.tile([P, CH], f32)
                        nc.tensor.matmul(
                            pj[:],
                            mmdt(rt[:]),
                            mmdt(g_sb[:, lo : lo + CH]),
                            start=True,
                            stop=True,
                        )
                        if j % 2 == 0:
                            nc.scalar.copy(ot[:, lo : lo + CH], pj[:])
                        else:
                            nc.vector.tensor_copy(ot[:, lo : lo + CH], pj[:])
                    nc.sync.dma_start(out[ts(t, P), :], ot[:])
                else:
                    CH = 420  # 3 chunks x 420 = 1260 per channel
                    ot = opool.tile([P, MOUT, C], f32)
                    k = 0
                    for c in range(C):
                        for j in range(MOUT // CH):
                            lo = j * CH
                            pj = pso.tile([P, CH], f32)
                            nc.tensor.matmul(
                                pj[:],
                                mmdt(rt[c * N : (c + 1) * N, :]),
                                mmdt(g_sb[c * N : (c + 1) * N, lo : lo + CH]),
                                start=True,
                                stop=True,
                            )
                            dst = ot[:, lo : lo + CH, c : c + 1]
                            if k % 2 == 0:
                                nc.scalar.copy(dst, pj[:])
                            else:
                                nc.vector.tensor_copy(dst, pj[:])
                            k += 1
                    nc.sync.dma_start(
                        out[ts(t, P), :], ot.rearrange("p a b -> p (a b)")
                    )

    nc.compile()
    return nc


_CACHE = {}


def _get(mode: str):
    if mode not in _CACHE:
        _CACHE[mode] = (_build_nc(mode), _g_const(mode), np.eye(P, dtype=np.float32))
    return _CACHE[mode]


def kernel(inputs: np.ndarray) -> np.ndarray:
    global LAST_RESULT
    assert inputs.shape == (B, N, C), inputs.shape
    nc, gconst, identity = _get(MODE)
    # host prep: x2[b, c*64+n] = inputs[b, n, c] (c-major for clean row groups)
    x2 = np.ascontiguousarray(
        np.asarray(inputs, dtype=np.float32).transpose(0, 2, 1).reshape(B, FIN)
    )
    in_maps = [
        {"x": x2[i * BPC : (i + 1) * BPC], "g": gconst, "ident": identity}
        for i in range(NCORES)
    ]
    res = run_bass_kernel_spmd(nc, in_maps, list(range(NCORES)), trace=TRACE)
    LAST_RESULT = res
    out = np.concatenate([res.results[i]["out"] for i in range(NCORES)], axis=0)
    return out.reshape(B, MOUT, 1, C)


# revision 2
# speedup vs baseline: 1.1184x; 1.1184x over previous
"""Trainium2 Bass kernel for nn_BSplineLayer (B-spline control-point solve + curve eval).

Key insight: the whole reference computation is LINEAR in the input radii r:
  Q = A @ r          (control-point solve: weighted sums + two first-order
                      linear recursions -> a dense 64x64 matrix A)
  curve = T @ Q      (closed cubic B-spline eval: per-segment gather of 4
                      control points x cubic basis -> sparse 1260x63 matrix T)
so  out[b, m, 0, c] = sum_n G[m, n] * r[b, n, c]  with  G = T @ A  (1260x64),
precomputed on the host in float64.

On device (per core, pure data parallel over batch):
  - load x tile [128 batch, 128] (c-major: col = c*64+n, host pre-permuted)
  - PE transpose -> rT [c*64+n, batch] in PSUM -> SBUF
  - PE matmul:  out[b, col] = rT.T @ GB  with GB [128, 2520] embedding G for
    both channels with interleaved output columns (col = 2m+c)
  - evacuate PSUM -> SBUF via ScalarE/VectorE, DMA out [128, 2520] contiguous.

The kernel is memory-bound: ~20.7 MB of HBM traffic per core (output dominates),
floor ~58 us at ~358 GB/s per-core HBM bandwidth.
"""

import os

import numpy as np

import concourse.bacc as bacc
import concourse.mybir as mybir
import concourse.tile as tile
from concourse.bass import ts
from concourse.bass_utils import run_bass_kernel_spmd

# Problem shape (hardcoded per contract: kernel.py is self-contained).
B, N, C = 16384, 64, 2
NCORES = 8
BPC = B // NCORES          # 2048 batch rows per core
P = 128                    # SBUF partitions
NTILES = BPC // P          # 16 batch tiles per core
NSEG = N - 1               # 63 segments
SAMP = 20                  # samples per segment
MOUT = NSEG * SAMP         # 1260 curve points
FIN = N * C                # 128 input floats per batch row
FOUT = MOUT * C            # 2520 output floats per batch row

# mode: "f32r_wide" (fp32 data, PE fp32r fast path, K=128 zero-interleaved G)
#       "fp32_wide" (exact fp32 matmul, 4x slower PE)
#       "fp32_packed" / "f32r_packed" (two K=64 row-group matmuls per tile)
MODE = os.environ.get("BSPLINE_MODE", "f32r_wide")
TRACE = bool(int(os.environ.get("BSPLINE_TRACE", "0")))

LAST_RESULT = None  # BassKernelResults of the most recent run (for test harness)


def _build_G(dtype=np.float64) -> np.ndarray:
    """G [1260, 64]: out[b, m, c] = sum_n G[m, n] * r[b, n, c]."""
    z1 = -2.0 + np.sqrt(np.asarray(3.0, dtype=dtype))
    powers = z1 ** np.arange(N, dtype=dtype)
    denom = 1.0 - z1**N
    # QT[i] as a linear functional of r (rows of a matrix); the *255/255
    # scaling in the reference cancels by linearity.
    QT = np.zeros((N, N), dtype=dtype)
    QT[0] = powers / denom
    for i in range(1, N):
        QT[i] = z1 * QT[i - 1]
        QT[i, i] += 1.0
    A = np.zeros((N, N), dtype=dtype)
    A[0] = -(6.0 * z1 / denom) * (powers[:, None] * QT).sum(axis=0)
    A[N - 1] = z1 * A[0] - 6.0 * z1 * QT[N - 1]
    for i in range(N - 2, 0, -1):
        A[i] = z1 * A[i + 1] - 6.0 * z1 * QT[i]
    # Cubic B-spline basis: curve[m=seg*20+s] = sum_k W[k, s] * Q[(seg+k) % 63]
    M = np.array(
        [
            [-1 / 6, 0.5, -0.5, 1 / 6],
            [0.5, -1.0, 0.5, 0.0],
            [-0.5, 0.0, 0.5, 0.0],
            [1 / 6, 2 / 3, 1 / 6, 0.0],
        ],
        dtype=dtype,
    )
    s = np.linspace(0.0, 1.0, SAMP).astype(dtype)
    S = np.stack([s**3, s**2, s, np.ones_like(s)], axis=0)
    W = M.T @ S  # [4, 20]
    G = np.zeros((MOUT, N), dtype=dtype)
    for seg in range(NSEG):
        for k in range(4):
            G[seg * SAMP : (seg + 1) * SAMP, :] += (
                W[k][:, None] * A[(seg + k) % NSEG][None, :]
            )
    return G


def _g_const(mode: str) -> np.ndarray:
    G = _build_G().astype(np.float32)
    if mode.endswith("wide"):
        # GB[c*64+n, 2m+c] = G[m, n]; zero elsewhere (K=128 single matmul).
        GB = np.zeros((P, FOUT), dtype=np.float32)
        for c in range(C):
            GB[c * N : (c + 1) * N, c::2] = G.T
        return GB
    # packed: GD[c*64+n, m] = G[m, n] (duplicated for both row groups).
    return np.concatenate([G.T, G.T], axis=0).astype(np.float32)


def _build_nc(mode: str):
    f32 = mybir.dt.float32
    f32r = mybir.dt.float32r
    # dtype of PE-facing data (DRAM params, SBUF input/weight tiles). The BIR
    # verifier requires every producer of an fp32r-matmult operand to emit
    # fp32r itself, so the whole pre-matmul chain is typed f32r in f32r mode.
    mdt = f32r if mode.startswith("f32r") else f32
    gcols = FOUT if mode.endswith("wide") else MOUT

    nc = bacc.Bacc("TRN2", target_bir_lowering=False, debug=False, num_devices=NCORES)
    x = nc.dram_tensor("x", [BPC, FIN], mdt, kind="ExternalInput").ap()
    g = nc.dram_tensor("g", [P, gcols], mdt, kind="ExternalInput").ap()
    ident = nc.dram_tensor("ident", [P, P], mdt, kind="ExternalInput").ap()
    out = nc.dram_tensor("out", [BPC, FOUT], f32, kind="ExternalOutput").ap()

    with tile.TileContext(nc) as tc:
        with (
            tc.tile_pool(name="const", bufs=1) as cpool,
            tc.tile_pool(name="xin", bufs=3) as xpool,
            tc.tile_pool(name="rt", bufs=2) as rpool,
            tc.tile_pool(name="outs", bufs=3) as opool,
            tc.tile_pool(name="pst", bufs=2, space="PSUM") as pst,
            tc.tile_pool(name="pso", bufs=6, space="PSUM") as pso,
        ):
            g_sb = cpool.tile([P, gcols], mdt)
            nc.sync.dma_start(g_sb[:], g[:])
            id_sb = cpool.tile([P, P], mdt)
            nc.sync.dma_start(id_sb[:], ident[:])

            for t in range(NTILES):
                xt = xpool.tile([P, FIN], mdt)
                nc.sync.dma_start(xt[:], x[ts(t, P), :])
                pt = pst.tile([P, P], mdt)
                nc.tensor.transpose(pt[:], xt[:], id_sb[:])
                rt = rpool.tile([P, P], mdt)
                nc.vector.tensor_copy(rt[:], pt[:])

                if mode.endswith("wide"):
                    CH = 504  # 5 chunks x 504 = 2520; one PSUM bank each
                    ot = opool.tile([P, FOUT], f32)
                    for j in range(FOUT // CH):
                        lo = j * CH
                        pj = pso.tile([P, CH], f32)
                        nc.tensor.matmul(
                            pj[:],
                            rt[:],
                            g_sb[:, lo : lo + CH],
                            start=True,
                            stop=True,
                        )
                        if j % 2 == 0:
                            nc.scalar.copy(ot[:, lo : lo + CH], pj[:])
                        else:
                            nc.vector.tensor_copy(ot[:, lo : lo + CH], pj[:])
                    nc.sync.dma_start(out[ts(t, P), :], ot[:])
                else:
                    CH = 420  # 3 chunks x 420 = 1260 per channel
                    ot = opool.tile([P, MOUT, C], f32)
                    k = 0
                    for c in range(C):
                        for j in range(MOUT // CH):
                            lo = j * CH
                            pj = pso.tile([P, CH], f32)
                            nc.tensor.matmul(
                                pj[:],
                                rt[c * N : (c + 1) * N, :],
                                g_sb[c * N : (c + 1) * N, lo : lo + CH],
                                start=True,
                                stop=True,
                            )
                            dst = ot[:, lo : lo + CH, c : c + 1]
                            if k % 2 == 0:
                                nc.scalar.copy(dst, pj[:])
                            else:
                                nc.vector.tensor_copy(dst, pj[:])
                            k += 1
                    nc.sync.dma_start(
                        out[ts(t, P), :], ot.rearrange("p a b -> p (a b)")
                    )

    nc.compile()
    return nc


_CACHE = {}


def _get(mode: str):
    if mode not in _CACHE:
        _CACHE[mode] = (_build_nc(mode), _g_const(mode), np.eye(P, dtype=np.float32))
    return _CACHE[mode]


def kernel(inputs: np.ndarray) -> np.ndarray:
    global LAST_RESULT
    assert inputs.shape == (B, N, C), inputs.shape
    nc, gconst, identity = _get(MODE)
    # host prep: x2[b, c*64+n] = inputs[b, n, c] (c-major for clean row groups)
    x2 = np.ascontiguousarray(
        np.asarray(inputs, dtype=np.float32).transpose(0, 2, 1).reshape(B, FIN)
    )
    in_maps = [
        {"x": x2[i * BPC : (i + 1) * BPC], "g": gconst, "ident": identity}
        for i in range(NCORES)
    ]
    res = run_bass_kernel_spmd(nc, in_maps, list(range(NCORES)), trace=TRACE)
    LAST_RESULT = res
    out = np.concatenate([res.results[i]["out"] for i in range(NCORES)], axis=0)
    return out.reshape(B, MOUT, 1, C)
